# revision 2
# baseline (speedup 1.0000x reference)
"""KNN top-k=16 Bass kernel for Trainium2, 8 NeuronCores.

Problem: query_points [4,4096,128] f32, sample_points [4,8192,128] f32, k=16.
Output: int32 indices [4,4096,16] of the k nearest samples per query
(ascending distance), matching jax.lax.top_k(-d, 16).

Sharding: core c handles batch b=c//2, query half h=c%2 (2048 queries/core),
with the full 8192-sample set for its batch. No cross-core communication.

Per-core pipeline, per 128-query tile (queries on partitions, samples free):
  score z = 2*q.s - |s|^2 (= -d + |q|^2; same ranking as -d)
  PE:  fp16 hi/lo split matmuls at 1 cycle/row (4x the fp32 path) with
       near-fp32 accuracy: x = xh + xl with xh = fp16(x), xl = fp16(x - xh);
       z ~= qh.sh + qh.sl + ql.sh (ql.sl term ~2^-22 relative, dropped),
       accumulated in fp32 PSUM on top of an -|s|^2 prefill (also fp16
       hi+lo, via two K=1 broadcast matmuls).
  ACT: evacuate PSUM -> z[128, 8192] in SBUF.
  DVE: per 512-chunk, max8 -> 8 candidate values, then max_index against
       the same chunk -> chunk-local positions (2 full scans total, vs 3
       for max8 + two whole-row max_index passes). Then top-16 of the 128
       candidates (max8 + match_replace + max8) and their positions in the
       candidate array (2 small max_index).
  GPSIMD: two local_scatter ops invert rank->cand-position into
       cand->rank and emit the 16 winning global indices in rank order.
"""

from contextlib import ExitStack

import numpy as np

import concourse.bass as bass
from concourse import bacc
import concourse.mybir as mybir
import concourse.tile as tile
from concourse.bass_utils import run_bass_kernel_spmd

B, N, M, D, K = 4, 4096, 8192, 128, 16
NCORES = 8
QPC = B * N // NCORES          # 2048 queries per core
NQT = QPC // 128               # 16 query tiles per core
CHUNK = 512                    # matmul / PSUM chunk (one bank)
NCH = M // CHUNK               # 16 chunks
NCAND = NCH * 8                # 128 candidates per query row
F32 = mybir.dt.float32
F16 = mybir.dt.float16
U16 = mybir.dt.uint16
I16 = mybir.dt.int16
I32 = mybir.dt.int32
NEG_INF = -3.0e38
Alu = mybir.AluOpType

_CACHE = {}


def build_nc(repeat=1):
    nc = bacc.Bacc("TRN2", target_bir_lowering=False, debug=False)
    q_d = nc.dram_tensor("q", [QPC, D], F32, kind="ExternalInput").ap()
    s_d = nc.dram_tensor("s", [M, D], F32, kind="ExternalInput").ap()
    ident_d = nc.dram_tensor("ident", [128, 128], F32, kind="ExternalInput").ap()
    negones_col_d = nc.dram_tensor("negones_col", [128, 1], F16, kind="ExternalInput").ap()
    ones_row_d = nc.dram_tensor("ones_row", [1, 128], F16, kind="ExternalInput").ap()
    out_d = nc.dram_tensor("out_idx", [QPC, K], U16, kind="ExternalOutput").ap()

    Copy = mybir.ActivationFunctionType.Copy
    Square = mybir.ActivationFunctionType.Square

    with tile.TileContext(nc) as tc, ExitStack() as ctx:
        const = ctx.enter_context(tc.tile_pool(name="const", bufs=1))
        big = ctx.enter_context(tc.tile_pool(name="big", bufs=1))
        ld = ctx.enter_context(tc.tile_pool(name="ld", bufs=4))
        zpool = ctx.enter_context(tc.tile_pool(name="z", bufs=2))
        small = ctx.enter_context(tc.tile_pool(name="small", bufs=2))

        ident = const.tile([128, 128], F32)
        nc.sync.dma_start(ident[:], ident_d[:])
        negones_colh = const.tile([128, 1], F16)
        nc.sync.dma_start(negones_colh[:], negones_col_d[:])
        ones_row = const.tile([1, 128], F16)
        nc.sync.dma_start(ones_row[:], ones_row_d[:])
        # rank1row[p, j] = j+1
        rank1row = const.tile([128, K], I16)
        nc.gpsimd.iota(rank1row[:], pattern=[[1, K]], base=1,
                       channel_multiplier=0)

        # persistent per-core SBUF arrays (hi/lo fp16 splits)
        STh = big.tile([128, M], F16)       # fp16(S^T)
        STl = big.tile([128, M], F16)       # fp16(S^T - STh)
        QTh = big.tile([128, QPC], F16)     # fp16((2Q)^T)
        QTl = big.tile([128, QPC], F16)     # fp16((2Q)^T - QTh)
        negs2h = big.tile([1, M], F16)      # fp16(-|s|^2)
        negs2l = big.tile([1, M], F16)      # fp16(-|s|^2 - negs2h)

        def body(main_ctx):
            psall = main_ctx.enter_context(tc.tile_pool(name="psall", bufs=1, space="PSUM"))

            # ---- preprocessing helpers (512-sample batches = 4 PE transposes) ----
            def prep_s_batch(bi):
                cs = slice(bi * CHUNK, (bi + 1) * CHUNK)
                ps = psall.tile([128, CHUNK], F32, tag="pst", bufs=1)
                for j in range(4):
                    t = bi * 4 + j
                    s_nat = ld.tile([128, D], F32, tag="snat", bufs=4)
                    nc.sync.dma_start(s_nat[:], s_d[t * 128:(t + 1) * 128, :])
                    nc.tensor.transpose(ps[:, j * 128:(j + 1) * 128], s_nat[:], ident[:])
                stf = ld.tile([128, CHUNK], F32, tag="stf", bufs=2)
                nc.scalar.activation(stf[:], ps[:], Copy)
                nc.scalar.activation(STh[:, cs], ps[:], Copy)
                # lo = fp16(s - hi) on the otherwise-idle Pool engine
                nc.gpsimd.tensor_tensor(STl[:, cs], stf[:], STh[:, cs], op=Alu.subtract)
                sqt = ld.tile([128, CHUNK], F32, tag="sqt", bufs=2)
                nc.scalar.activation(sqt[:], ps[:], Square)
                sqth = ld.tile([128, CHUNK], F16, tag="sqth", bufs=2)
                nc.scalar.activation(sqth[:], sqt[:], Copy)
                sqtl = ld.tile([128, CHUNK], F16, tag="sqtl", bufs=2)
                nc.gpsimd.tensor_tensor(sqtl[:], sqt[:], sqth[:], op=Alu.subtract)
                ps2 = psall.tile([1, CHUNK], F32, tag="ps2", bufs=1)
                nc.tensor.matmul(ps2[:], negones_colh[:], sqth[:], start=True, stop=False)
                nc.tensor.matmul(ps2[:], negones_colh[:], sqtl[:], start=False, stop=True)
                s2f = ld.tile([1, CHUNK], F32, tag="s2f", bufs=2)
                nc.scalar.activation(s2f[:], ps2[:], Copy)
                nc.scalar.activation(negs2h[:, cs], ps2[:], Copy)
                nc.gpsimd.tensor_tensor(negs2l[:, cs], s2f[:], negs2h[:, cs], op=Alu.subtract)

            def prep_q_batch(bi):
                cs = slice(bi * CHUNK, (bi + 1) * CHUNK)
                ps = psall.tile([128, CHUNK], F32, tag="pst", bufs=1)
                for j in range(4):
                    t = bi * 4 + j
                    q_nat = ld.tile([128, D], F32, tag="qnat", bufs=4)
                    nc.sync.dma_start(q_nat[:], q_d[t * 128:(t + 1) * 128, :])
                    nc.tensor.transpose(ps[:, j * 128:(j + 1) * 128], q_nat[:], ident[:])
                qtf = ld.tile([128, CHUNK], F32, tag="qtf", bufs=2)
                nc.scalar.activation(qtf[:], ps[:], Copy, scale=2.0)
                nc.scalar.activation(QTh[:, cs], ps[:], Copy, scale=2.0)
                nc.gpsimd.tensor_tensor(QTl[:, cs], qtf[:], QTh[:, cs], op=Alu.subtract)

            # ---- main-loop helpers ----
            def qt_pairs(qt, z, cands, clocal, pairs):
                qs = slice(qt * 128, (qt + 1) * 128)
                for p in pairs:
                    psm = psall.tile([128, 2 * CHUNK], F32, tag="psm", bufs=3)
                    hs = [(psm[:, h * CHUNK:(h + 1) * CHUNK],
                           slice((2 * p + h) * CHUNK, (2 * p + h + 1) * CHUNK))
                          for h in range(2)]
                    # ordered by stationary: ones_row, QTh, QTl
                    for half, ss in hs:
                        nc.tensor.matmul(half, ones_row[:], negs2h[:, ss],
                                         start=True, stop=False)
                        nc.tensor.matmul(half, ones_row[:], negs2l[:, ss],
                                         start=False, stop=False)
                    for half, ss in hs:
                        nc.tensor.matmul(half, QTh[:, qs], STh[:, ss],
                                         start=False, stop=False)
                        nc.tensor.matmul(half, QTh[:, qs], STl[:, ss],
                                         start=False, stop=False)
                    for half, ss in hs:
                        nc.tensor.matmul(half, QTl[:, qs], STh[:, ss],
                                         start=False, stop=True)
                    nc.scalar.activation(z[:, 2 * p * CHUNK:(2 * p + 2) * CHUNK],
                                         psm[:], Copy)
                    for h in range(2):
                        ch = 2 * p + h
                        zc = z[:, ch * CHUNK:(ch + 1) * CHUNK]
                        nc.vector.max(out=cands[:, ch * 8:ch * 8 + 8], in_=zc)
                        nc.vector.max_index(out=clocal[:, ch * 8:ch * 8 + 8],
                                            in_max=cands[:, ch * 8:ch * 8 + 8],
                                            in_values=zc)

            def qt_finish(qt, z, cands, clocal):
                # top-16 of the 128 candidates + their candidate positions
                m1 = small.tile([128, 8], F32, tag="m1")
                crep = small.tile([128, NCAND], F32, tag="crep")
                m2 = small.tile([128, 8], F32, tag="m2")
                pos = small.tile([128, K], U16, tag="pos")
                nc.vector.max(out=m1[:], in_=cands[:])
                nc.vector.match_replace(out=crep[:], in_to_replace=m1[:],
                                        in_values=cands[:], imm_value=NEG_INF)
                nc.vector.max(out=m2[:], in_=crep[:])
                nc.vector.max_index(out=pos[:, 0:8], in_max=m1[:], in_values=cands[:])
                nc.vector.max_index(out=pos[:, 8:16], in_max=m2[:], in_values=crep[:])
                # GPSIMD rank->index assembly: rkarr[p, pos[p, j]] = j+1
                rkarr = small.tile([128, NCAND], I16, tag="rkarr")
                nc.gpsimd.local_scatter(rkarr[:], rank1row[:], pos[:].bitcast(I16),
                                        channels=128, num_elems=NCAND, num_idxs=K)
                # idxs2 = rkarr - 1: rank for winner slots, -1 (ignored) elsewhere
                idxs2 = small.tile([128, NCAND], I16, tag="idxs2")
                nc.gpsimd.tensor_scalar_sub(idxs2[:], rkarr[:], 1)
                # outl[p, rank] = chunk-local index of the rank'th winner
                outl = small.tile([128, K], U16, tag="outl")
                nc.gpsimd.local_scatter(outl[:], clocal[:], idxs2[:],
                                        channels=128, num_elems=K, num_idxs=NCAND)
                # global index = local + 512*(pos//8): (pos & ~7) << 6
                base16 = small.tile([128, K], U16, tag="base16")
                nc.vector.tensor_scalar(base16[:], pos[:], 0xFFF8, 6,
                                        op0=Alu.bitwise_and,
                                        op1=Alu.logical_shift_left)
                out16 = small.tile([128, K], U16, tag="out16")
                nc.vector.tensor_tensor(out16[:], outl[:], base16[:], op=Alu.add)
                nc.sync.dma_start(out_d[qt * 128:(qt + 1) * 128, :], out16[:])

            def qt_tiles():
                z = zpool.tile([128, M], F32, tag="z")
                cands = small.tile([128, NCAND], F32, tag="cands")
                clocal = small.tile([128, NCAND], U16, tag="clocal")
                return z, cands, clocal

            # ---- emission: interleave preprocessing with qt0/qt1 so no
            # engine sits idle waiting for the serial transpose phase.
            # qt0's pair p only needs S batches 2p,2p+1 and q batch 0. ----
            for bi in range(6):
                prep_s_batch(bi)
            prep_q_batch(0)
            z0, cands0, clocal0 = qt_tiles()
            for p in range(8):
                qt_pairs(0, z0, cands0, clocal0, [p])
                for bi in (2 * p + 6, 2 * p + 7):
                    if bi < NCH:
                        prep_s_batch(bi)
            qt_finish(0, z0, cands0, clocal0)
            z1, cands1, clocal1 = qt_tiles()
            for p in range(8):
                qt_pairs(1, z1, cands1, clocal1, [p])
                if p < NQT // 4 - 1:
                    prep_q_batch(p + 1)
            qt_finish(1, z1, cands1, clocal1)
            for qt in range(2, NQT):
                z, cands, clocal = qt_tiles()
                qt_pairs(qt, z, cands, clocal, range(8))
                qt_finish(qt, z, cands, clocal)

        for _ in range(repeat):
            with ExitStack() as main_ctx:
                body(main_ctx)
    nc.compile()
    return nc


def build_null_nc():
    """Same external I/O as the real kernel, but no compute: isolates
    PJRT dispatch + host<->HBM transfer overhead for timing."""
    nc = bacc.Bacc("TRN2", target_bir_lowering=False, debug=False)
    nc.dram_tensor("q", [QPC, D], F32, kind="ExternalInput").ap()
    nc.dram_tensor("s", [M, D], F32, kind="ExternalInput").ap()
    ident_d = nc.dram_tensor("ident", [128, 128], F32, kind="ExternalInput").ap()
    nc.dram_tensor("negones_col", [128, 1], F16, kind="ExternalInput").ap()
    nc.dram_tensor("ones_row", [1, 128], F16, kind="ExternalInput").ap()
    out_d = nc.dram_tensor("out_idx", [QPC, K], U16, kind="ExternalOutput").ap()
    with tile.TileContext(nc) as tc, ExitStack() as ctx:
        pool = ctx.enter_context(tc.tile_pool(name="sb", bufs=1))
        t = pool.tile([128, 16], F32)
        nc.sync.dma_start(t[:], ident_d[:, 0:16])
        ti = pool.tile([128, 16], U16)
        nc.vector.tensor_copy(ti[:], t[:])
        for qt in range(NQT):
            nc.sync.dma_start(out_d[qt * 128:(qt + 1) * 128, :], ti[:])
    nc.compile()
    return nc


def _consts():
    return {
        "ident": np.eye(128, dtype=np.float32),
        "negones_col": np.full((128, 1), -1.0, np.float16),
        "ones_row": np.ones((1, 128), np.float16),
    }


def kernel(query_points, sample_points, k, **run_kwargs):
    assert int(k) == K
    q = np.ascontiguousarray(np.asarray(query_points), dtype=np.float32)
    s = np.ascontiguousarray(np.asarray(sample_points), dtype=np.float32)
    if "nc" not in _CACHE:
        _CACHE["nc"] = build_nc()
    nc = _CACHE["nc"]
    consts = _consts()
    in_maps = []
    for c in range(NCORES):
        b, h = c // 2, c % 2
        in_maps.append(dict(
            q=q[b, h * QPC:(h + 1) * QPC, :],
            s=s[b],
            **consts,
        ))
    res = run_bass_kernel_spmd(nc, in_maps, list(range(NCORES)), **run_kwargs)
    out = np.empty((B, N, K), np.int32)
    for c in range(NCORES):
        b, h = c // 2, c % 2
        out[b, h * QPC:(h + 1) * QPC, :] = res.results[c]["out_idx"].astype(np.int32)
    return out


if __name__ == "__main__":
    rng = np.random.default_rng(0)
    qp = rng.standard_normal((B, N, D), dtype=np.float32)
    sp = rng.standard_normal((B, M, D), dtype=np.float32)
    idx = kernel(qp, sp, K)
    print(idx.shape, idx.dtype, idx[0, 0])


# revision 3
# speedup vs baseline: 1.0578x; 1.0578x over previous
"""KNN top-k=16 Bass kernel for Trainium2, 8 NeuronCores.

Problem: query_points [4,4096,128] f32, sample_points [4,8192,128] f32, k=16.
Output: int32 indices [4,4096,16] of the k nearest samples per query
(ascending distance), matching jax.lax.top_k(-d, 16).

Sharding: core c handles batch b=c//2, query half h=c%2 (2048 queries/core),
with the full 8192-sample set for its batch. No cross-core communication.

Per-core pipeline, per 128-query tile (queries on partitions, samples free):
  score z = 2*q.s - |s|^2 (= -d + |q|^2; same ranking as -d)
  PE:  fp16 hi/lo split matmuls at 1 cycle/row (4x the fp32 path) with
       near-fp32 accuracy: x = xh + xl with xh = fp16(x), xl = fp16(x - xh);
       z ~= qh.sh + qh.sl + ql.sh (ql.sl term ~2^-22 relative, dropped),
       accumulated in fp32 PSUM on top of an -|s|^2 prefill (also fp16
       hi+lo, via two K=1 broadcast matmuls).
  ACT: evacuate PSUM -> z[128, 8192] in SBUF.
  DVE: per 512-chunk, max8 -> 8 candidate values, then max_index against
       the same chunk -> chunk-local positions (2 full scans total, vs 3
       for max8 + two whole-row max_index passes). Then top-16 of the 128
       candidates (max8 + match_replace + max8) and their positions in the
       candidate array (2 small max_index).
  GPSIMD: two local_scatter ops invert rank->cand-position into
       cand->rank and emit the 16 winning global indices in rank order.
"""

from contextlib import ExitStack

import numpy as np

import concourse.bass as bass
from concourse import bacc
import concourse.mybir as mybir
import concourse.tile as tile
from concourse.bass_utils import run_bass_kernel_spmd

B, N, M, D, K = 4, 4096, 8192, 128, 16
NCORES = 8
QPC = B * N // NCORES          # 2048 queries per core
NQT = QPC // 128               # 16 query tiles per core
CHUNK = 512                    # matmul / PSUM chunk (one bank)
NCH = M // CHUNK               # 16 chunks
NCAND = NCH * 8                # 128 candidates per query row
F32 = mybir.dt.float32
F16 = mybir.dt.float16
U16 = mybir.dt.uint16
I16 = mybir.dt.int16
I32 = mybir.dt.int32
NEG_INF = -3.0e38
Alu = mybir.AluOpType

_CACHE = {}


def build_nc(repeat=1):
    nc = bacc.Bacc("TRN2", target_bir_lowering=False, debug=False)
    q_d = nc.dram_tensor("q", [QPC, D], F32, kind="ExternalInput").ap()
    s_d = nc.dram_tensor("s", [M, D], F32, kind="ExternalInput").ap()
    ident_d = nc.dram_tensor("ident", [128, 128], F32, kind="ExternalInput").ap()
    negones_col_d = nc.dram_tensor("negones_col", [128, 1], F16, kind="ExternalInput").ap()
    ones_row_d = nc.dram_tensor("ones_row", [1, 128], F16, kind="ExternalInput").ap()
    out_d = nc.dram_tensor("out_idx", [QPC, K], U16, kind="ExternalOutput").ap()

    Copy = mybir.ActivationFunctionType.Copy
    Square = mybir.ActivationFunctionType.Square

    with tile.TileContext(nc) as tc, ExitStack() as ctx:
        const = ctx.enter_context(tc.tile_pool(name="const", bufs=1))
        big = ctx.enter_context(tc.tile_pool(name="big", bufs=1))
        ld = ctx.enter_context(tc.tile_pool(name="ld", bufs=4))
        zpool = ctx.enter_context(tc.tile_pool(name="z", bufs=3))
        small = ctx.enter_context(tc.tile_pool(name="small", bufs=3))

        ident = const.tile([128, 128], F32)
        nc.sync.dma_start(ident[:], ident_d[:])
        negones_colh = const.tile([128, 1], F16)
        nc.sync.dma_start(negones_colh[:], negones_col_d[:])
        ones_row = const.tile([1, 128], F16)
        nc.sync.dma_start(ones_row[:], ones_row_d[:])
        # rank1row[p, j] = j+1
        rank1row = const.tile([128, K], I16)
        nc.gpsimd.iota(rank1row[:], pattern=[[1, K]], base=1,
                       channel_multiplier=0)

        # persistent per-core SBUF arrays (hi/lo fp16 splits)
        STh = big.tile([128, M], F16)       # fp16(S^T)
        STl = big.tile([128, M], F16)       # fp16(S^T - STh)
        QTh = big.tile([128, QPC], F16)     # fp16((2Q)^T)
        QTl = big.tile([128, QPC], F16)     # fp16((2Q)^T - QTh)
        negs2h = big.tile([1, M], F16)      # fp16(-|s|^2)
        negs2l = big.tile([1, M], F16)      # fp16(-|s|^2 - negs2h)

        def body(main_ctx):
            psall = main_ctx.enter_context(tc.tile_pool(name="psall", bufs=1, space="PSUM"))

            # ---- preprocessing helpers (512-sample batches = 4 PE transposes) ----
            def prep_s_batch(bi):
                cs = slice(bi * CHUNK, (bi + 1) * CHUNK)
                ps = psall.tile([128, CHUNK], F32, tag="pst", bufs=1)
                for j in range(4):
                    t = bi * 4 + j
                    s_nat = ld.tile([128, D], F32, tag="snat", bufs=4)
                    nc.sync.dma_start(s_nat[:], s_d[t * 128:(t + 1) * 128, :])
                    nc.tensor.transpose(ps[:, j * 128:(j + 1) * 128], s_nat[:], ident[:])
                stf = ld.tile([128, CHUNK], F32, tag="stf", bufs=2)
                nc.scalar.activation(stf[:], ps[:], Copy)
                nc.scalar.activation(STh[:, cs], ps[:], Copy)
                # lo = fp16(s - hi) on the otherwise-idle Pool engine
                nc.gpsimd.tensor_tensor(STl[:, cs], stf[:], STh[:, cs], op=Alu.subtract)
                sqt = ld.tile([128, CHUNK], F32, tag="sqt", bufs=2)
                nc.scalar.activation(sqt[:], ps[:], Square)
                sqth = ld.tile([128, CHUNK], F16, tag="sqth", bufs=2)
                nc.scalar.activation(sqth[:], sqt[:], Copy)
                sqtl = ld.tile([128, CHUNK], F16, tag="sqtl", bufs=2)
                nc.gpsimd.tensor_tensor(sqtl[:], sqt[:], sqth[:], op=Alu.subtract)
                ps2 = psall.tile([1, CHUNK], F32, tag="ps2", bufs=1)
                nc.tensor.matmul(ps2[:], negones_colh[:], sqth[:], start=True, stop=False)
                nc.tensor.matmul(ps2[:], negones_colh[:], sqtl[:], start=False, stop=True)
                s2f = ld.tile([1, CHUNK], F32, tag="s2f", bufs=2)
                nc.scalar.activation(s2f[:], ps2[:], Copy)
                nc.scalar.activation(negs2h[:, cs], ps2[:], Copy)
                nc.gpsimd.tensor_tensor(negs2l[:, cs], s2f[:], negs2h[:, cs], op=Alu.subtract)

            def prep_q_batch(bi):
                cs = slice(bi * CHUNK, (bi + 1) * CHUNK)
                ps = psall.tile([128, CHUNK], F32, tag="pst", bufs=1)
                for j in range(4):
                    t = bi * 4 + j
                    q_nat = ld.tile([128, D], F32, tag="qnat", bufs=4)
                    nc.sync.dma_start(q_nat[:], q_d[t * 128:(t + 1) * 128, :])
                    nc.tensor.transpose(ps[:, j * 128:(j + 1) * 128], q_nat[:], ident[:])
                qtf = ld.tile([128, CHUNK], F32, tag="qtf", bufs=2)
                nc.scalar.activation(qtf[:], ps[:], Copy, scale=2.0)
                nc.scalar.activation(QTh[:, cs], ps[:], Copy, scale=2.0)
                nc.gpsimd.tensor_tensor(QTl[:, cs], qtf[:], QTh[:, cs], op=Alu.subtract)

            # ---- main-loop helpers ----
            def qt_pairs(qt, z, cands, clocal, pairs):
                qs = slice(qt * 128, (qt + 1) * 128)
                for p in pairs:
                    hs = []
                    for h in range(2):
                        psm = psall.tile([128, CHUNK], F32, tag="psm", bufs=6)
                        hs.append((psm,
                                   slice((2 * p + h) * CHUNK, (2 * p + h + 1) * CHUNK)))
                    # ordered by stationary: ones_row, QTh, QTl (weight
                    # reloads are expensive on HW: keep 3 loads per pair)
                    for half, ss in hs:
                        nc.tensor.matmul(half[:], ones_row[:], negs2h[:, ss],
                                         start=True, stop=False)
                        nc.tensor.matmul(half[:], ones_row[:], negs2l[:, ss],
                                         start=False, stop=False)
                    for half, ss in hs:
                        nc.tensor.matmul(half[:], QTh[:, qs], STh[:, ss],
                                         start=False, stop=False)
                        nc.tensor.matmul(half[:], QTh[:, qs], STl[:, ss],
                                         start=False, stop=False)
                    for half, ss in hs:
                        nc.tensor.matmul(half[:], QTl[:, qs], STh[:, ss],
                                         start=False, stop=True)
                    for h in range(2):
                        ch = 2 * p + h
                        half = hs[h][0]
                        zc = z[:, ch * CHUNK:(ch + 1) * CHUNK]
                        nc.scalar.activation(zc, half[:], Copy)
                        nc.vector.max(out=cands[:, ch * 8:ch * 8 + 8], in_=zc)
                        nc.vector.max_index(out=clocal[:, ch * 8:ch * 8 + 8],
                                            in_max=cands[:, ch * 8:ch * 8 + 8],
                                            in_values=zc)

            def qt_finish(qt, z, cands, clocal):
                # top-16 of the 128 candidates + their candidate positions
                m1 = small.tile([128, 8], F32, tag="m1")
                crep = small.tile([128, NCAND], F32, tag="crep")
                m2 = small.tile([128, 8], F32, tag="m2")
                pos = small.tile([128, K], U16, tag="pos")
                nc.vector.max(out=m1[:], in_=cands[:])
                nc.vector.match_replace(out=crep[:], in_to_replace=m1[:],
                                        in_values=cands[:], imm_value=NEG_INF)
                nc.vector.max(out=m2[:], in_=crep[:])
                nc.vector.max_index(out=pos[:, 0:8], in_max=m1[:], in_values=cands[:])
                nc.vector.max_index(out=pos[:, 8:16], in_max=m2[:], in_values=crep[:])
                # GPSIMD rank->index assembly: rkarr[p, pos[p, j]] = j+1
                rkarr = small.tile([128, NCAND], I16, tag="rkarr")
                nc.gpsimd.local_scatter(rkarr[:], rank1row[:], pos[:].bitcast(I16),
                                        channels=128, num_elems=NCAND, num_idxs=K)
                # idxs2 = rkarr - 1: rank for winner slots, -1 (ignored) elsewhere
                idxs2 = small.tile([128, NCAND], I16, tag="idxs2")
                nc.gpsimd.tensor_scalar_sub(idxs2[:], rkarr[:], 1)
                # outl[p, rank] = chunk-local index of the rank'th winner
                outl = small.tile([128, K], U16, tag="outl")
                nc.gpsimd.local_scatter(outl[:], clocal[:], idxs2[:],
                                        channels=128, num_elems=K, num_idxs=NCAND)
                # global index = local + 512*(pos//8): (pos & ~7) << 6
                base16 = small.tile([128, K], U16, tag="base16")
                nc.vector.tensor_scalar(base16[:], pos[:], 0xFFF8, 6,
                                        op0=Alu.bitwise_and,
                                        op1=Alu.logical_shift_left)
                out16 = small.tile([128, K], U16, tag="out16")
                nc.vector.tensor_tensor(out16[:], outl[:], base16[:], op=Alu.add)
                nc.sync.dma_start(out_d[qt * 128:(qt + 1) * 128, :], out16[:])

            def qt_tiles():
                z = zpool.tile([128, M], F32, tag="z")
                cands = small.tile([128, NCAND], F32, tag="cands")
                clocal = small.tile([128, NCAND], U16, tag="clocal")
                return z, cands, clocal

            # ---- emission: interleave preprocessing with qt0/qt1 so no
            # engine sits idle waiting for the serial transpose phase.
            # qt0's pair p only needs S batches 2p,2p+1 and q batch 0. ----
            for bi in range(6):
                prep_s_batch(bi)
            prep_q_batch(0)
            z0, cands0, clocal0 = qt_tiles()
            for p in range(8):
                qt_pairs(0, z0, cands0, clocal0, [p])
                for bi in (2 * p + 6, 2 * p + 7):
                    if bi < NCH:
                        prep_s_batch(bi)
            qt_finish(0, z0, cands0, clocal0)
            z1, cands1, clocal1 = qt_tiles()
            for p in range(8):
                qt_pairs(1, z1, cands1, clocal1, [p])
                if p < NQT // 4 - 1:
                    prep_q_batch(p + 1)
            qt_finish(1, z1, cands1, clocal1)
            for qt in range(2, NQT):
                z, cands, clocal = qt_tiles()
                qt_pairs(qt, z, cands, clocal, range(8))
                qt_finish(qt, z, cands, clocal)

        for _ in range(repeat):
            with ExitStack() as main_ctx:
                body(main_ctx)
    nc.compile()
    return nc


def build_null_nc():
    """Same external I/O as the real kernel, but no compute: isolates
    PJRT dispatch + host<->HBM transfer overhead for timing."""
    nc = bacc.Bacc("TRN2", target_bir_lowering=False, debug=False)
    nc.dram_tensor("q", [QPC, D], F32, kind="ExternalInput").ap()
    nc.dram_tensor("s", [M, D], F32, kind="ExternalInput").ap()
    ident_d = nc.dram_tensor("ident", [128, 128], F32, kind="ExternalInput").ap()
    nc.dram_tensor("negones_col", [128, 1], F16, kind="ExternalInput").ap()
    nc.dram_tensor("ones_row", [1, 128], F16, kind="ExternalInput").ap()
    out_d = nc.dram_tensor("out_idx", [QPC, K], U16, kind="ExternalOutput").ap()
    with tile.TileContext(nc) as tc, ExitStack() as ctx:
        pool = ctx.enter_context(tc.tile_pool(name="sb", bufs=1))
        t = pool.tile([128, 16], F32)
        nc.sync.dma_start(t[:], ident_d[:, 0:16])
        ti = pool.tile([128, 16], U16)
        nc.vector.tensor_copy(ti[:], t[:])
        for qt in range(NQT):
            nc.sync.dma_start(out_d[qt * 128:(qt + 1) * 128, :], ti[:])
    nc.compile()
    return nc


def _consts():
    return {
        "ident": np.eye(128, dtype=np.float32),
        "negones_col": np.full((128, 1), -1.0, np.float16),
        "ones_row": np.ones((1, 128), np.float16),
    }


def kernel(query_points, sample_points, k, **run_kwargs):
    assert int(k) == K
    q = np.ascontiguousarray(np.asarray(query_points), dtype=np.float32)
    s = np.ascontiguousarray(np.asarray(sample_points), dtype=np.float32)
    if "nc" not in _CACHE:
        _CACHE["nc"] = build_nc()
    nc = _CACHE["nc"]
    consts = _consts()
    in_maps = []
    for c in range(NCORES):
        b, h = c // 2, c % 2
        in_maps.append(dict(
            q=q[b, h * QPC:(h + 1) * QPC, :],
            s=s[b],
            **consts,
        ))
    res = run_bass_kernel_spmd(nc, in_maps, list(range(NCORES)), **run_kwargs)
    out = np.empty((B, N, K), np.int32)
    for c in range(NCORES):
        b, h = c // 2, c % 2
        out[b, h * QPC:(h + 1) * QPC, :] = res.results[c]["out_idx"].astype(np.int32)
    return out


if __name__ == "__main__":
    rng = np.random.default_rng(0)
    qp = rng.standard_normal((B, N, D), dtype=np.float32)
    sp = rng.standard_normal((B, M, D), dtype=np.float32)
    idx = kernel(qp, sp, K)
    print(idx.shape, idx.dtype, idx[0, 0])
